# revision 3
# baseline (speedup 1.0000x reference)
"""CrossDomainInterestLoss on 8 Trainium2 NeuronCores — v2.

Strategy (hardcoded for bs=4096, dim=128), 4x2 core grid as v1, but the
per-sim-element work is restructured around the bf16-bit-pattern trick:

  - PE: sim group [128, 2048] fp32 in PSUM (4 matmuls, f32r, 4 banks).
  - ACT path (most tiles): exp(sim/tau) -> bf16 e_t in SBUF with fused
    fp32 row-sum accum (pos). The bf16 BIT PATTERN of e_t, viewed as
    int16, is affine in sim (bits ~ k*s + c +- mantissa wobble), so the
    HNM stats come from two stock 4x-mode tensor_scalar ops on the view:
      cnt  = accum(is_gt(bits, theta))
      rsum = (accum(max(bits, theta)) - N*theta) / k
  - DVE path (a few tiles, to offload ACT): bits = K*sim + C computed by
    one 1x tensor_scalar straight from PSUM into an int16 tile (fast-exp
    bit trick); the same tile viewed as bf16 holds approx exp values ->
    4x accum gives pos; is_gt/max on the int16 view give HNM stats.
  - Host: gather partials, remove the diagonal in the bits domain
    (replicating device quantization), apply the reference formula.
"""

import numpy as np

import concourse.bass as bass
import concourse.mybir as mybir
from concourse import bacc, tile
from concourse.bass_utils import run_bass_kernel_spmd

TAU = 0.05
HARD_NEG_WEIGHT = 0.5
MARGIN = 0.3
BS = 4096
DIM = 128

R, C = 4, 2           # row-groups x col-groups = 8 cores
ROWS = BS // R        # u rows per core (1024)
COLS = BS // C        # negative rows per core per matrix (2048)
NRC = ROWS // 128     # 128-row chunks per core (8)

F32 = mybir.dt.float32
F32R = mybir.dt.float32r
BF16 = mybir.dt.bfloat16
I16 = mybir.dt.int16

LOG2E = 1.4426950408889634
K_BITS = 128.0 / (TAU * np.log(2.0))     # d bits / d sim  (3693.59)

# DVE-path fast-exp: bits = round(K_BITS*s + C_DVE). C centered so the
# geometric-mean of val(bits)/exp(s/tau) over the mantissa wobble is 1;
# +0.5 assumes truncation on the fp32->int16 convert.
# E[log2(1+x) - x] over x~U[0,1) = 2 - 1/ln2 - 1/2 ~= 0.0573
_MEAN_LOG2_WOBBLE = 2.0 - 1.0 / np.log(2.0) - 0.5

# Exponent shift: the device computes exp(s/tau - BETA) so the bf16 bit
# patterns land in [~200, 8400]. Sum-of-max accumulations then stay below
# 2^24 and are EXACT in fp32 (the unshifted sums at ~3.5e7 pick up
# hundreds of ulps of systematic rounding). Host rescales pos by
# 2^(SHIFT/128).
SHIFT = 11648.0
BETA = SHIFT * np.log(2.0) / 128.0
POS_SCALE = 2.0 ** (SHIFT / 128.0)

C_DVE = 128.0 * (127.0 - _MEAN_LOG2_WOBBLE) + 0.5 - SHIFT  # +0.5: i16 trunc
CEFF_DVE = C_DVE - 0.5  # E[bits] = K*s + CEFF
# mx threshold must be an integer (keeps every accum term integral).
TH_MX_DVE = float(np.round(K_BITS * MARGIN + C_DVE))
TH_GT_DVE = TH_MX_DVE - 0.63  # counts bits >= TH_MX

# ACT path: bits = bf16bits(exp(s/tau - BETA)) = K_BITS*s + 128*(127 -
# BETA*log2e - w(frac)), w = log2(1+x)-x in [0, .0861], mean ~0.0573.
CEFF_ACT = 128.0 * (127.0 - _MEAN_LOG2_WOBBLE) - SHIFT


def _bits_at_margin():
    import ml_dtypes

    v = np.float32(np.exp(MARGIN / TAU - BETA))
    return float(np.asarray(v).astype(ml_dtypes.bfloat16).view(np.uint16))


# Count/relu boundary at the exact device bits of exp(m/tau - BETA).
TH_MX_ACT = _bits_at_margin()
TH_GT_ACT = TH_MX_ACT - 0.63

# tile indices (rc*2 + m) handled by the DVE fast-exp path
DVE_TILES = frozenset((5, 10, 15))

_BUILT = None
LAST_RESULTS = None
TRACE = False
REPS = 1
DYN_REPS = 0


def _build_bass():
    nc = bacc.Bacc()

    ut = nc.dram_tensor("ut", [DIM, ROWS], F32R, kind="ExternalInput")
    at = nc.dram_tensor("at", [DIM, COLS], F32R, kind="ExternalInput")
    bt = nc.dram_tensor("bt", [DIM, COLS], F32R, kind="ExternalInput")

    outs = {}
    for name in ("pos_a", "pos_b", "gt_a", "gt_b", "mx_a", "mx_b"):
        outs[name] = nc.dram_tensor(name, [128, NRC], F32, kind="ExternalOutput")

    with tile.TileContext(nc) as tc:
        with (
            tc.tile_pool(name="ops", bufs=1) as ops,
            tc.tile_pool(name="stats", bufs=1) as stats,
            tc.tile_pool(name="escr", bufs=2) as escr,
            tc.tile_pool(name="bscr", bufs=2) as bscr,
            tc.tile_pool(name="jscr", bufs=2) as jscr,
            tc.tile_pool(name="kscr", bufs=2) as kscr,
            tc.tile_pool(name="psum", bufs=2, space=bass.MemorySpace.PSUM) as psum,
        ):
            ut_s = ops.tile([DIM, ROWS], F32R, tag="ut")
            at_s = ops.tile([DIM, COLS], F32R, tag="at")
            bt_s = ops.tile([DIM, COLS], F32R, tag="bt")
            half = COLS // 2
            nc.gpsimd.dma_start(ut_s[:], ut[:])
            nc.sync.dma_start(at_s[:, :512], at[:, :512])
            nc.sync.dma_start(at_s[:, 512:half], at[:, 512:half])
            nc.sync.dma_start(at_s[:, half:], at[:, half:])
            nc.sync.dma_start(bt_s[:, :half], bt[:, :half])
            nc.sync.dma_start(bt_s[:, half:], bt[:, half:])

            st = {n: stats.tile([128, NRC], F32, tag=n, name=n) for n in outs}
            neg_beta = stats.tile([128, 1], F32, tag="neg_beta")
            nc.gpsimd.memset(neg_beta[:], -BETA)
            # Dummy exp so LoadActFuncSet overlaps the input DMAs.
            warm = stats.tile([128, 1], F32, tag="warm", name="warm")
            nc.scalar.activation(
                warm[:],
                nc.const_aps.tensor(0.0, (128, 1), F32),
                mybir.ActivationFunctionType.Exp,
            )
            neg = {0: at_s, 1: bt_s}
            sfx = {0: "a", 1: "b"}

            def body():
                for rc in range(NRC):
                    lhsT = ut_s[:, rc * 128 : (rc + 1) * 128]
                    for m in (0, 1):
                        t_idx = rc * 2 + m
                        col = slice(rc, rc + 1)
                        sim = psum.tile([128, COLS], F32, tag="sim", name="sim")
                        for n in range(COLS // 512):
                            nc.tensor.matmul(
                                sim[:, n * 512 : (n + 1) * 512],
                                lhsT,
                                neg[m][:, n * 512 : (n + 1) * 512],
                                start=True,
                                stop=True,
                            )
                        if t_idx in DVE_TILES:
                            bits = bscr.tile([128, COLS], I16, tag="bits", name="bits")
                            nc.vector.tensor_scalar(
                                bits[:], sim[:], K_BITS, C_DVE,
                                mybir.AluOpType.mult, mybir.AluOpType.add,
                            )
                            iv = bits[:]
                            bv = bits[:].bitcast(BF16)
                            jt = jscr.tile([128, COLS], BF16, tag="j", name="j")
                            nc.vector.tensor_scalar(
                                jt[:], bv, 0.0, None,
                                mybir.AluOpType.add, mybir.AluOpType.add,
                                accum_out=st["pos_" + sfx[m]][:, col],
                            )
                            th_gt, th_mx = TH_GT_DVE, TH_MX_DVE
                        else:
                            e_t = escr.tile([128, COLS], BF16, tag="e", name="e")
                            nc.scalar.activation(
                                e_t[:],
                                sim[:],
                                mybir.ActivationFunctionType.Exp,
                                scale=1.0 / TAU,
                                bias=neg_beta[:],
                                accum_out=st["pos_" + sfx[m]][:, col],
                            )
                            iv = e_t[:].bitcast(I16)
                            th_gt, th_mx = TH_GT_ACT, TH_MX_ACT
                        kt = kscr.tile([128, COLS], I16, tag="k", name="k")
                        nc.vector.tensor_scalar(
                            kt[:], iv, th_gt, None,
                            mybir.AluOpType.is_gt, mybir.AluOpType.add,
                            accum_out=st["gt_" + sfx[m]][:, col],
                        )
                        kt2 = kscr.tile([128, COLS], I16, tag="k", name="k")
                        nc.vector.tensor_scalar(
                            kt2[:], iv, th_mx, None,
                            mybir.AluOpType.max, mybir.AluOpType.add,
                            accum_out=st["mx_" + sfx[m]][:, col],
                        )

            if DYN_REPS > 0:
                with tc.For_i(0, DYN_REPS, 1):
                    body()
            else:
                for _rep in range(REPS):
                    body()

            for name, dram in outs.items():
                nc.sync.dma_start(dram[:], st[name][:])

    nc.compile()
    return nc


def _get_built():
    global _BUILT
    if _BUILT is None:
        _BUILT = _build_bass()
    return _BUILT


def _l2norm(x):
    n = np.linalg.norm(x.astype(np.float64), axis=1, keepdims=True)
    return (x.astype(np.float64) / np.maximum(n, 1e-12)).astype(np.float32)


def _round_f32r(x):
    import ml_dtypes

    hi = x.astype(ml_dtypes.bfloat16).astype(np.float32)
    lo = (x - hi).astype(ml_dtypes.bfloat16).astype(np.float32)
    return hi + lo


def _bits_of(vals_f64):
    """bf16 bit pattern (as int) of exp-like positive fp32 values, matching
    the device ACT-path quantization."""
    import ml_dtypes

    return (
        np.asarray(vals_f64, np.float32)
        .astype(ml_dtypes.bfloat16)
        .view(np.uint16)
        .astype(np.float64)
    )


def gather_partials(results):
    def gather(name):
        out = np.zeros(BS, dtype=np.float64)
        for k in range(8):
            rg = k // C
            arr = results[k][name].astype(np.float64)  # [128, NRC]
            blk = arr.T.reshape(ROWS)
            out[rg * ROWS : (rg + 1) * ROWS] += blk
        return out

    return {n: gather(n) for n in ("pos_a", "pos_b", "gt_a", "gt_b", "mx_a", "mx_b")}


def kernel(user_interest, reg_A_emb, reg_B_emb):
    global LAST_RESULTS
    u = _l2norm(np.asarray(user_interest, dtype=np.float32))
    a = _l2norm(np.asarray(reg_A_emb, dtype=np.float32))
    b = _l2norm(np.asarray(reg_B_emb, dtype=np.float32))

    u = _round_f32r(u)
    a = _round_f32r(a)
    b = _round_f32r(b)
    in_maps = []
    for k in range(8):
        rg, cg = k // C, k % C
        in_maps.append(
            {
                "ut": np.ascontiguousarray(u[rg * ROWS : (rg + 1) * ROWS].T),
                "at": np.ascontiguousarray(a[cg * COLS : (cg + 1) * COLS].T),
                "bt": np.ascontiguousarray(b[cg * COLS : (cg + 1) * COLS].T),
            }
        )

    nc = _get_built()
    res = run_bass_kernel_spmd(nc, in_maps, list(range(8)), trace=TRACE)
    LAST_RESULTS = res

    g = gather_partials(res.results)
    pos_A, pos_B = g["pos_a"], g["pos_b"]
    cnt = {"a": g["gt_a"], "b": g["gt_b"]}
    # mx accum = sum over the tile's 2048 cols of max(bits, th); both cores
    # (cg=0,1) contribute, each over 2048 cols -> subtract 2*2048*th total.
    rsb = {"a": g["mx_a"], "b": g["mx_b"]}

    pos_A = pos_A * POS_SCALE
    pos_B = pos_B * POS_SCALE

    rows = np.arange(BS)
    rc_of = rows % ROWS // 128
    t_of = {"a": rc_of * 2, "b": rc_of * 2 + 1}
    th_of, ceff_of, is_dve = {}, {}, {}
    for mkey in ("a", "b"):
        is_dve[mkey] = np.isin(t_of[mkey], list(DVE_TILES))
        thg = np.where(is_dve[mkey], TH_GT_DVE, TH_GT_ACT)
        th_of[mkey] = np.where(is_dve[mkey], TH_MX_DVE, TH_MX_ACT)
        ceff_of[mkey] = np.where(is_dve[mkey], CEFF_DVE, CEFF_ACT)
        rsb[mkey] = rsb[mkey] - 2 * COLS * th_of[mkey]

    # Diagonal removal in the bits domain, replicating device quantization.
    u64, a64, b64 = u.astype(np.float64), a.astype(np.float64), b.astype(np.float64)
    d = {"a": np.sum(u64 * a64, axis=1), "b": np.sum(u64 * b64, axis=1)}
    for mkey in ("a", "b"):
        s = d[mkey]
        bits_act = _bits_of(np.exp(s / TAU - BETA))
        bits_dve = np.floor(K_BITS * s + C_DVE)
        bits = np.where(is_dve[mkey], bits_dve, bits_act)
        thg = np.where(is_dve[mkey], TH_GT_DVE, TH_GT_ACT)
        cnt[mkey] = cnt[mkey] - (bits > thg)
        rsb[mkey] = rsb[mkey] - np.maximum(bits - th_of[mkey], 0.0)

    denom = np.maximum(pos_A + pos_B, 1e-9)
    loss_A = -np.mean(np.log(pos_A / denom))
    loss_B = -np.mean(np.log(pos_B / denom))
    base_loss = (loss_A + loss_B) / 2.0

    def hnm(mkey):
        # row_sum = sum_{sel j} s_j = (sum_{sel} bits - cnt*ceff)/K
        #         = (rsb + cnt*(th - ceff))/K
        cnt_m, rs = cnt[mkey], rsb[mkey]
        has = cnt_m > 0.5
        n_rows = np.count_nonzero(has)
        if n_rows == 0:
            return 0.0
        row_tot = (rs + cnt_m * (th_of[mkey] - ceff_of[mkey])) / K_BITS
        return np.sum(row_tot[has]) / n_rows

    weighted_hard = 0.5 * hnm("a") + 1.0 * hnm("b")
    total = base_loss + (
        HARD_NEG_WEIGHT * weighted_hard if abs(weighted_hard) > 1e-9 else 0.0
    )
    return np.float32(total)
